# revision 22
# baseline (speedup 1.0000x reference)
"""Distributed Bass kernel for tied-row MSA attention on 8 TRN2 NeuronCores.

Sharding: batch (64 MSA rows) split 8 ways; weights/attn_bias replicated.
The tie_dim mean over q is computed REDUNDANTLY per core from a replicated
copy of x (no collective): host ships the full transposed x to every core
(own rows first); the row-sum is a DVE fold tree overlapped with the DMA.

Per-core dataflow (all-transposed, bf16 matmuls, f32 softmax):
  xT [dp | dc | tok] loaded pre-transposed from host (bf16)
  kT = Wk^T xT, gT = sigmoid(Wg^T xT + bg), v = xT^T Wv   (local 2048 tok)
  rsT = sum over all 64 rows of xT (DVE folds) ; q_tT = Wq^T rsT * scale
  ebias = exp(biasT)  (ACT, one-time);  mg = mask_i*g ; ugg = (1-mask_i)*u*g
  per (r,jc,hg): mega[j, hp, i] = k_h^T q_h  (2 concurrent K=64 row-tiles,
                 block-diag q pairs; 32/96 row positions unsupported on HW)
                 ae = exp(mega - 30*(1-mask_j)) * ebias   (ACT + DVE)
  per (r,hg):    bankV = v^T ae ; bankS = ones^T ae  (col-tiled PE)
                 og = (bankV / bankS) * mg
  out = og^T Wo + ugg^T Wo + 1^T bo  (all fused in one PSUM group per row)
"""

import numpy as np
import ml_dtypes

_bf16 = ml_dtypes.bfloat16

HEADS = 8
DH = 32
B = 64
N = 256
D = 256
INNER = 256
NCORES = 8
RLOC = B // NCORES          # 8 MSA rows per core
T = RLOC * N                # 2048 local tokens per core
SCALE_F = 1.0 / (B * (DH ** 0.5))  # tie-mean (1/64) * dh^-0.5, folded into q
MASK_NEG = 30.0             # pre-softmax mask offset

_CACHE = {}


def _build():
    import concourse.bass as bass
    import concourse.mybir as mybir
    import concourse.tile as tile
    from concourse import bacc
    from contextlib import ExitStack

    f32 = mybir.dt.float32
    bf16 = mybir.dt.bfloat16
    AF = mybir.ActivationFunctionType
    ALU = mybir.AluOpType

    nc = bacc.Bacc("TRN2", target_bir_lowering=False, debug=False,
                   num_devices=NCORES, num_swdge_queues=4)

    xT_e = nc.dram_tensor("xT", [128, B, 2, N], bf16, kind="ExternalInput")
    biasT_e = nc.dram_tensor("biasT", [128, 2, HEADS, N], bf16, kind="ExternalInput")
    maskT_e = nc.dram_tensor("maskT", [128, 2 * RLOC], f32, kind="ExternalInput")
    mfull_e = nc.dram_tensor("mfull", [128, T], bf16, kind="ExternalInput")
    wq_e = nc.dram_tensor("WqT", [128, 2, INNER], bf16, kind="ExternalInput")
    wk_e = nc.dram_tensor("WkT", [128, 2, INNER], bf16, kind="ExternalInput")
    wv_e = nc.dram_tensor("WvT", [128, 2, INNER], bf16, kind="ExternalInput")
    wg_e = nc.dram_tensor("WgT", [128, 2, INNER], bf16, kind="ExternalInput")
    wo_e = nc.dram_tensor("WoT", [128, 2, D], bf16, kind="ExternalInput")
    bg_e = nc.dram_tensor("bg", [128, 2], f32, kind="ExternalInput")
    bo_e = nc.dram_tensor("bo", [1, D], bf16, kind="ExternalInput")
    out_e = nc.dram_tensor("out", [RLOC, 128, 2, D], bf16, kind="ExternalOutput")

    with tile.TileContext(nc) as tc, ExitStack() as ctx:
        const = ctx.enter_context(tc.tile_pool(name="const", bufs=1))
        big = ctx.enter_context(tc.tile_pool(name="big", bufs=1))
        rspool = ctx.enter_context(tc.tile_pool(name="rs", bufs=1))
        work = ctx.enter_context(tc.tile_pool(name="work", bufs=3))
        aepool = ctx.enter_context(tc.tile_pool(name="ae", bufs=10))
        ogpool = ctx.enter_context(tc.tile_pool(name="og", bufs=4))
        ps_mega = ctx.enter_context(tc.tile_pool(name="ps_mega", bufs=2, space="PSUM"))
        ps_av = ctx.enter_context(tc.tile_pool(name="ps_av", bufs=2, space="PSUM"))
        ps_u = ctx.enter_context(tc.tile_pool(name="ps_u", bufs=1, space="PSUM"))
        ps_fp = ctx.enter_context(tc.tile_pool(name="ps_fp", bufs=1, space="PSUM"))

        # ---- DMAs: local x rows (gpsimd queue), consts (sync), remote x ----
        xT = big.tile([128, B, 2, N], bf16)
        for p in range(8):  # local 8 rows, 1-row pieces on two issue queues
            eng = nc.gpsimd if p % 2 == 0 else nc.scalar
            eng.dma_start(out=xT[:, p:p + 1, :, :],
                          in_=xT_e.ap()[:, p:p + 1, :, :])
        wq_sb = const.tile([128, 2, INNER], bf16)
        wk_sb = const.tile([128, 2, INNER], bf16)
        wv_sb = const.tile([128, 2, INNER], bf16)
        wg_sb = const.tile([128, 2, INNER], bf16)
        wo_sb = const.tile([128, 2, D], bf16)
        nc.sync.dma_start(out=wk_sb, in_=wk_e.ap())
        nc.sync.dma_start(out=wg_sb, in_=wg_e.ap())
        nc.sync.dma_start(out=wv_sb, in_=wv_e.ap())
        nc.sync.dma_start(out=wq_sb, in_=wq_e.ap())
        nc.sync.dma_start(out=wo_sb, in_=wo_e.ap())
        bg_sb = const.tile([128, 2], f32)
        nc.sync.dma_start(out=bg_sb, in_=bg_e.ap())
        bo_row = const.tile([1, D], bf16)
        nc.sync.dma_start(out=bo_row, in_=bo_e.ap())
        maskT = const.tile([128, 2 * RLOC], f32)
        nc.sync.dma_start(out=maskT, in_=maskT_e.ap())
        mfull = const.tile([128, T], bf16)
        nc.sync.dma_start(out=mfull, in_=mfull_e.ap())
        biasT = const.tile([128, 2, HEADS, N], bf16)
        for p in range(4):
            nc.sync.dma_start(out=biasT[:, :, 2 * p:2 * (p + 1), :],
                              in_=biasT_e.ap()[:, :, 2 * p:2 * (p + 1), :])
        for p in range(14):  # remote 56 rows, 4-row pieces, 3 issue queues
            eng = (nc.gpsimd, nc.sync, nc.scalar)[p % 3]
            eng.dma_start(out=xT[:, 8 + 4 * p:12 + 4 * p, :, :],
                          in_=xT_e.ap()[:, 8 + 4 * p:12 + 4 * p, :, :])

        # ---- tiny consts ----
        ones32 = const.tile([128, 32], bf16)
        nc.vector.memset(ones32, 1.0)
        onesc = const.tile([128, 1], bf16)
        nc.vector.memset(onesc, 1.0 / N)
        ones1 = const.tile([1, 128], bf16)
        nc.vector.memset(ones1, 1.0)
        qbd2 = big.tile([128, 2, 2, N], bf16)
        nc.vector.memset(qbd2, 0.0)
        bg_half = const.tile([128, 2], f32)
        nc.vector.tensor_scalar_mul(bg_half, bg_sb, 0.5)
        # maskbias[j, (r,jc)] = -30*(1-mask_j)  (per-partition exp bias)
        maskbias = const.tile([128, 2 * RLOC], f32)
        nc.vector.tensor_scalar(maskbias, maskT, MASK_NEG, -MASK_NEG,
                                ALU.mult, ALU.add)
        # ---- projections on local rows (k, g transposed; v natural) ----
        kT = big.tile([128, 2, T], bf16)
        gT = big.tile([128, 2, T], bf16)
        for mc in range(2):
            for t8 in range(2):
                gp = ps_mega.tile([128, 4, 256], f32, name="mega")
                gpf = gp.rearrange("p a b -> p (a b)")
                for q in range(2):
                    for kc in range(2):
                        nc.tensor.matmul(
                            gpf[:, 512 * q:512 * (q + 1)],
                            wg_sb[:, kc, 128 * mc:128 * (mc + 1)],
                            xT[:, t8 * 4 + 2 * q:t8 * 4 + 2 * q + 2, kc, :],
                            start=(kc == 0), stop=(kc == 1))
                nc.scalar.activation(gT[:, mc, 1024 * t8:1024 * (t8 + 1)],
                                     gpf, AF.Tanh,
                                     bias=bg_half[:, mc:mc + 1], scale=0.5)
            for t8 in range(2):
                kp = ps_mega.tile([128, 4, 256], f32, name="mega")
                kpf = kp.rearrange("p a b -> p (a b)")
                for q in range(2):
                    for kc in range(2):
                        nc.tensor.matmul(
                            kpf[:, 512 * q:512 * (q + 1)],
                            wk_sb[:, kc, 128 * mc:128 * (mc + 1)],
                            xT[:, t8 * 4 + 2 * q:t8 * 4 + 2 * q + 2, kc, :],
                            start=(kc == 0), stop=(kc == 1))
                nc.scalar.activation(
                    kT[:, mc, 1024 * t8:1024 * (t8 + 1)], kpf, AF.Copy)
        v_nat = big.tile([128, 16, INNER], bf16)
        for tp in range(8):  # token-tile pairs packed into one bank
            vp = ps_mega.tile([128, 4, 256], f32, name="mega")
            vpf = vp.rearrange("p a b -> p (a b)")
            for ti in range(2):  # one group of 4 in bank0
                t = 2 * tp + ti
                for kc in range(2):
                    nc.tensor.matmul(
                        vpf[:, 256 * ti:256 * (ti + 1)],
                        xT[:, t // 2, kc, 128 * (t % 2):128 * (t % 2) + 128],
                        wv_sb[:, kc, :],
                        start=(ti == 0 and kc == 0), stop=(ti == 1 and kc == 1))
            nc.scalar.activation(
                v_nat[:, 2 * tp:2 * tp + 2, :].rearrange("p a b -> p (a b)"),
                vpf[:, 0:512], AF.Copy)

        # ---- ebias = exp(biasT) (ACT; also loads the exp table set) ----
        ebias = const.tile([128, 2, HEADS, N], bf16)
        for jc in range(2):
            nc.scalar.activation(ebias[:, jc], biasT[:, jc], AF.Exp)

        # ---- uniform rows u[hd, hg, r] = sum_j v / 256 (one PSUM group) ----
        ups = ps_u.tile([128, 2, N], f32, name="ups")
        for r in range(RLOC):
            for hg in range(2):
                for jc in range(2):
                    nc.tensor.matmul(
                        ups[:, hg, r:r + 1],
                        v_nat[:, 2 * r + jc, 128 * hg:128 * (hg + 1)],
                        onesc,
                        start=(r == 0 and hg == 0 and jc == 0),
                        stop=(r == RLOC - 1 and hg == 1 and jc == 1))
        u_all = const.tile([128, 2, RLOC, 1], f32)
        nc.vector.tensor_copy(u_all[:, :, :, 0], ups[:, :, 0:RLOC])

        # ---- row-sum of all 64 rows (chunk folds, bf16 accumulator so
        # every add stays in 2x perf mode; q-side rounding is ~100x damped) ----
        rsT = big.tile([128, 2, N], bf16)
        for k in range(8):
            r0 = 8 * k
            c4 = rspool.tile([128, 4, 2, N], bf16, tag="c4")
            nc.vector.tensor_add(c4, xT[:, r0:r0 + 4, :, :],
                                 xT[:, r0 + 4:r0 + 8, :, :])
            c2 = rspool.tile([128, 2, 2, N], bf16, tag="c2")
            nc.vector.tensor_add(c2, c4[:, 0:2, :, :], c4[:, 2:4, :, :])
            if k == 0:
                nc.vector.tensor_add(rsT, c2[:, 0, :, :], c2[:, 1, :, :])
            else:
                c1 = rspool.tile([128, 2, N], bf16, tag="c1")
                nc.vector.tensor_add(c1, c2[:, 0, :, :], c2[:, 1, :, :])
                nc.vector.tensor_add(rsT, rsT, c1)

        # ---- q_tiedT = Wq^T rsT * scale, then block-diag K=64 pairs ----
        q_tT = big.tile([128, 2, N], bf16)
        for mc in range(2):
            qp = ps_u.tile([128, 2, N], f32, name="ups")
            for kc in range(2):
                nc.tensor.matmul(qp[:, 0, :], wq_sb[:, kc, 128 * mc:128 * (mc + 1)],
                                 rsT[:, kc, :], start=(kc == 0), stop=(kc == 1))
            nc.vector.tensor_scalar_mul(q_tT[:, mc, :], qp[:, 0, :], SCALE_F)
        for hg in range(2):
            for hp in range(4):
                nc.vector.tensor_copy(
                    qbd2[32 * hp:32 * (hp + 1), hg, hp % 2, :],
                    q_tT[32 * hp:32 * (hp + 1), hg, :])

        # ---- blend precomputes (2x/4x-mode): sig, mg = m*sig,
        # ugg[hg,r] = (sig-mg)*u  (per-partition AP scalar multiplies) ----
        sig = big.tile([128, 2, T], bf16)
        nc.vector.tensor_scalar(sig, gT, 0.5, 0.5, ALU.mult, ALU.add)
        mg = big.tile([128, 2, T], bf16)
        for hg in range(2):
            nc.vector.tensor_mul(mg[:, hg], sig[:, hg], mfull)
        isig = big.tile([128, 2, T], bf16)
        nc.vector.tensor_sub(isig, sig, mg)
        ugg = big.tile([128, 2, RLOC, N], bf16)
        for hg in range(2):
            for r in range(RLOC):
                nc.vector.tensor_scalar_mul(
                    ugg[:, hg, r], isig[:, hg, r * N:(r + 1) * N],
                    u_all[:, hg, r])

        # ---- attention stages ----
        def stage_dots(r, jc):
            out = {}
            for hg in range(2):
                mega = ps_mega.tile([128, 4, 256], f32, name="mega")
                megaf = mega.rearrange("p a b -> p (a b)")
                for t2 in range(2):
                    nc.tensor.matmul(
                        megaf[:, 512 * t2:512 * (t2 + 1)],
                        kT[64 * t2:64 * (t2 + 1), hg,
                           r * N + 128 * jc:r * N + 128 * (jc + 1)],
                        qbd2[64 * t2:64 * (t2 + 1), hg, :, :].rearrange(
                            "p a b -> p (a b)"),
                        start=True, stop=True,
                        tile_position=(64 * t2, 0))
                ae = aepool.tile([128, 4, 256], bf16, tag="ae")
                nc.scalar.activation(ae.rearrange("p a b -> p (a b)"),
                                     megaf, AF.Exp,
                                     bias=maskbias[:, 2 * r + jc:2 * r + jc + 1],
                                     scale=1.0)
                nc.vector.tensor_mul(ae, ae, ebias[:, jc, 4 * hg:4 * (hg + 1), :])
                out[hg] = ae
            return out

        def stage_av(r, hg, aes):
            bvs = ps_av.tile([128, 2, 256], f32, name="bankVS")
            for hp in range(4):
                h = 4 * hg + hp
                orow = slice(32 * hp, 32 * (hp + 1))
                # V and S share the bank on these partitions -> one group of 4
                for jc in range(2):
                    rhs = aes[jc][hg][:, hp, :]
                    nc.tensor.matmul(
                        bvs[orow, 0, :],
                        v_nat[:, 2 * r + jc, 32 * h:32 * (h + 1)],
                        rhs, start=(jc == 0), stop=False,
                        tile_position=(0, 32 * hp))
                    nc.tensor.matmul(
                        bvs[orow, 1, :], ones32, rhs,
                        start=False, stop=(jc == 1),
                        tile_position=(0, 32 * hp))
            return bvs

        def stage_og(r, hg, bvs):
            rc = work.tile([128, 256], f32, tag="rc")
            nc.vector.reciprocal_approx_fast(out=rc, in_=bvs[:, 1, :])
            og = ogpool.tile([128, 256], bf16, tag="og")
            nc.vector.tensor_mul(og, bvs[:, 0, :], rc)
            nc.vector.tensor_mul(og, og, mg[:, hg, r * N:(r + 1) * N])
            return og

        def stage_wo(r, ogs):
            # one PSUM group per row: og@Wo + ones^T bo, both ic halves
            fp = ps_fp.tile([128, 2, 256], f32, name="fp")
            for ic in range(2):
                nc.tensor.matmul(fp[:, ic, :],
                                 ogs[0][:, 128 * ic:128 * (ic + 1)],
                                 wo_sb[:, 0, :], start=(ic == 0), stop=False)
                nc.tensor.matmul(fp[:, ic, :],
                                 ogs[1][:, 128 * ic:128 * (ic + 1)],
                                 wo_sb[:, 1, :], start=False, stop=False)
                nc.tensor.matmul(fp[:, ic, :],
                                 ugg[:, 0, r, 128 * ic:128 * (ic + 1)],
                                 wo_sb[:, 0, :], start=False, stop=False)
                nc.tensor.matmul(fp[:, ic, :],
                                 ugg[:, 1, r, 128 * ic:128 * (ic + 1)],
                                 wo_sb[:, 1, :], start=False, stop=False)
                nc.tensor.matmul(fp[:, ic, :], ones1, bo_row,
                                 start=False, stop=(ic == 1))
            fo = work.tile([128, 2, D], bf16, tag="fo")
            nc.scalar.activation(fo, fp, AF.Copy)
            nc.sync.dma_start(out=out_e[r], in_=fo)

        # ---- software pipeline over rows (wo lags one extra row) ----
        aes_prev = {0: stage_dots(0, 0), 1: stage_dots(0, 1)}
        ogs_prev = None
        for r in range(1, RLOC + 2):
            aes_next = {}
            if r < RLOC:
                aes_next[0] = stage_dots(r, 0)
            if r <= RLOC:
                bvs0 = stage_av(r - 1, 0, aes_prev)
            if r < RLOC:
                aes_next[1] = stage_dots(r, 1)
            if r <= RLOC:
                bvs1 = stage_av(r - 1, 1, aes_prev)
            if ogs_prev is not None:
                stage_wo(r - 2, ogs_prev)
            if r <= RLOC:
                og0 = stage_og(r - 1, 0, bvs0)
                og1 = stage_og(r - 1, 1, bvs1)
                ogs_prev = {0: og0, 1: og1}
                aes_prev = aes_next
                if r == RLOC:
                    stage_wo(r - 1, ogs_prev)
                    ogs_prev = None

    nc.finalize()
    return nc


def _get_nc():
    if "nc" not in _CACHE:
        _CACHE["nc"] = _build()
    return _CACHE["nc"]


def _in_maps(x, mask, attn_bias, Wq, Wkv, Wg, bg, Wo, bo):
    x = np.asarray(x, dtype=np.float32)
    mask = np.asarray(mask)
    attn_bias = np.asarray(attn_bias, dtype=np.float32)
    Wq = np.asarray(Wq, dtype=np.float32)
    Wkv = np.asarray(Wkv, dtype=np.float32)
    Wg = np.asarray(Wg, dtype=np.float32)
    bg = np.asarray(bg, dtype=np.float32)
    Wo = np.asarray(Wo, dtype=np.float32)
    bo = np.asarray(bo, dtype=np.float32)

    # xT[dp, r, dc, n] = x[r, n, dc*128+dp]
    xT = np.ascontiguousarray(
        x.transpose(2, 0, 1).reshape(2, 128, B, N).transpose(1, 2, 0, 3)
    ).astype(_bf16)
    # biasT[jp, jc, h, i] = bias[h, i, jc*128+jp]
    biasT = np.ascontiguousarray(
        attn_bias.reshape(HEADS, N, N).transpose(2, 0, 1)
        .reshape(2, 128, HEADS, N).transpose(1, 0, 2, 3)
    ).astype(_bf16)

    def wlay(W):  # [256, out] -> [p, kc, out]
        return np.ascontiguousarray(
            W.reshape(2, 128, W.shape[1]).transpose(1, 0, 2)).astype(_bf16)

    shared = {
        "biasT": biasT,
        "WqT": wlay(Wq),
        "WkT": wlay(Wkv[:, 0:INNER]),
        "WvT": wlay(Wkv[:, INNER:2 * INNER]),
        "WgT": wlay(Wg),
        "WoT": wlay(Wo),
        "bg": np.ascontiguousarray(bg.reshape(2, 128).T),
        "bo": np.ascontiguousarray(bo.reshape(1, D)).astype(_bf16),
    }
    maps = []
    order = np.arange(B).reshape(NCORES, RLOC)
    for c in range(NCORES):
        rows = np.concatenate([order[c], np.delete(order, c, axis=0).ravel()])
        m = dict(shared)
        m["xT"] = np.ascontiguousarray(xT[:, rows, :, :])
        lm = mask[order[c]]  # [8, 256] local rows
        m["maskT"] = np.ascontiguousarray(
            lm.reshape(RLOC, 2, 128).transpose(2, 0, 1).reshape(128, 2 * RLOC)
        ).astype(np.float32)
        m["mfull"] = np.ascontiguousarray(
            np.broadcast_to(lm.reshape(1, T), (128, T))
        ).astype(np.float32).astype(_bf16)
        maps.append(m)
    return maps


def kernel(x, mask, attn_bias, Wq, Wkv, Wg, bg, Wo, bo, tie_dim=64, **_unused):
    from concourse.bass_utils import run_bass_kernel_spmd

    nc = _get_nc()
    in_maps = _in_maps(x, mask, attn_bias, Wq, Wkv, Wg, bg, Wo, bo)
    res = run_bass_kernel_spmd(nc, in_maps, core_ids=list(range(NCORES)))
    outs = []
    for c in range(NCORES):
        o = np.asarray(res.results[c]["out"], dtype=np.float32)  # [8,128,2,256]
        outs.append(o.transpose(0, 2, 1, 3).reshape(RLOC, N, D))
    return np.concatenate(outs, axis=0)

